# revision 61
# baseline (speedup 1.0000x reference)
"""DenseCapsLayer Trainium2 kernel.

Math (per (n, a) pair; A=32 input capsule types, B=32 output, P=4, hw=256):
  votes v[h,b] = W[a,b] @ M[h]  (4x4 matmuls) -- NEVER materialized (256MB).
  Routing reduces to small per-pair contractions:
    Mbar[b]   = sum_h c[h,b] * M[h]          (c = softmax over h of L)
    S[b]      = W[a,b] @ Mbar[b]
    n2[b]     = |S[b]|^2 = <Mbar[b], G[a,b] @ Mbar[b]>,  G = W^T W  (host-precomputed)
    Pout[b]   = f(n2) * S[b]                  (squash factor f)
    U[b]      = W^T Pout[b] = f * G @ Mbar[b]
    L        += M @ U^T  (so L_t = M @ Ubar_t^T with Ubar = cumulative sum of U)
  Final output = Pout at iter 2.

Sharding: data-parallel over batch: core c handles n in {2c, 2c+1} (NL=2), all
32 a's. Per-core layout: 16 "groups" g = j*2 + nl (j = a-block of 4, nl =
local n); partitions = (aL, b) = aL*32 + b with aL = a - 4j.
"""

import numpy as np
import ml_dtypes

import concourse.bass as bass
import concourse.bacc as bacc
import concourse.mybir as mybir
import concourse.tile as tile
from concourse.bass_utils import run_bass_kernel_spmd

F32 = mybir.dt.float32
F16 = mybir.dt.float16
BF16 = mybir.dt.bfloat16

A, B, P, ITERS = 32, 32, 4, 3
PS = P * P                      # 16
BATCH, OH, OW = 16, 16, 16
HW = OH * OW                    # 256
NCORES = 8
NL = BATCH // NCORES            # 2 local batch items per core
J = A // 4                      # 8 groups of 4 a's
G = J * NL                      # 16 (g = j*NL + nl)
NB = 4                          # g-batches for L/exp processing (4 g each)
EPS = 1e-8

AF = mybir.ActivationFunctionType
ALU = mybir.AluOpType
AX = mybir.AxisListType


# ---------------------------------------------------------------- device code
import os as _os
_STOP = _os.environ.get("K_STOP", "")


def _emit(tc, xs16t, xh16, xl16, wga, wws, o32):
    nc = tc.nc

    dbg_view = o32.rearrange("n a b k -> (n a b k)") \
                  .rearrange("(p f) -> p f", f=256)

    def dump(src):
        # debug: copy a (128, 256) fp32 AP to the output
        nc.sync.dma_start(out=dbg_view, in_=src)

    with (
        tc.tile_pool(name="inp", bufs=1) as inp,
        tc.tile_pool(name="state", bufs=1) as state,
        tc.tile_pool(name="work", bufs=2) as work,
        tc.tile_pool(name="small", bufs=2) as small,
        tc.tile_pool(name="lps", bufs=2, space="PSUM") as lps_pool,
        tc.tile_pool(name="mbps", bufs=1, space="PSUM") as mbps_pool,
        tc.tile_pool(name="dram", bufs=2, space="DRAM") as dram,
    ):
        # ---------------- persistent inputs in SBUF (batched DMAs)
        Xh = {}
        Xl = {}
        for ch in range(2):
            th = inp.tile([128, NL * A * PS], BF16, tag=f"xh{ch}")
            nc.sync.dma_start(
                out=th[:].rearrange("p (n c) -> p n c", n=NL),
                in_=xh16[:, ch * 128:(ch + 1) * 128, :].rearrange(
                    "n p c -> p n c"))
            tl = inp.tile([128, NL * A * PS], BF16, tag=f"xl{ch}")
            nc.sync.dma_start(
                out=tl[:].rearrange("p (n c) -> p n c", n=NL),
                in_=xl16[:, ch * 128:(ch + 1) * 128, :].rearrange(
                    "n p c -> p n c"))
            for nl in range(NL):
                Xh[nl, ch] = th[:, nl * A * PS:(nl + 1) * A * PS]
                Xl[nl, ch] = tl[:, nl * A * PS:(nl + 1) * A * PS]

        GA = inp.tile([128, G * 64], F16, tag="ga")
        nc.scalar.dma_start(out=GA[:], in_=wga[:, :])
        WS = inp.tile([128, G * 64], F32, tag="ws")
        nc.scalar.dma_start(out=WS[:], in_=wws[:, :])

        # MTall: (kq, g*1024 + aL*256 + h) fp16 -- M^T pre-transposed on the
        # HOST (static input), one DMA.  All matmul operands must live at
        # partition base 0 in this environment (mixing PE row-groups faults).
        MTall = inp.tile([PS, G * 4 * HW], F16, tag="mtall")
        nc.sync.dma_start(
            out=MTall[:].rearrange("p (g c) -> p g c", g=G),
            in_=xs16t.rearrange("g p c -> p g c"))
        MT16 = {g: MTall[:, g * 4 * HW:(g + 1) * 4 * HW] for g in range(G)}

        ones_bf = inp.tile([128, 128], BF16, tag="ones_bf")
        nc.gpsimd.memset(ones_bf[:], 1.0)
        onecol = inp.tile([128, 1], BF16, tag="onecol")
        nc.gpsimd.memset(onecol[:], 1.0)
        epsc = inp.tile([128, 1], F32, tag="epsc")
        nc.gpsimd.memset(epsc[:], EPS)

        # Preload the combined exp+ln activation table set once; otherwise the
        # table-load pass alternates exp_and_others / natural_log every iter
        # (~1.3us per reload).
        from concourse.hw_specs import get_activation_tables
        _tables = list(get_activation_tables(nc.m.arch).items())
        _set_id = next(i for i, (nm, fns) in enumerate(_tables)
                       if AF.Exp in fns and AF.Ln in fns)
        nc.scalar.add_instruction(mybir.InstLoadActFuncSet(
            name=nc.get_next_instruction_name(),
            ins=[], outs=[], act_func_set_id=_set_id))

        if _STOP == "setup":
            dump(WS[:, 0:256])
            return

        ubar_prev = None
        lps_tiles = {}

        for t in range(ITERS):
            # -------- Mb matmuls (+ exp for t>0), processed in 4-g batches
            mb_ps0 = mbps_pool.tile([128, 8 * 64], F32, tag="mb0")
            mb_ps1 = mbps_pool.tile([128, 8 * 64], F32, tag="mb1")
            mb_ps = [mb_ps0, mb_ps1]
            den_ps = None
            if t > 0:
                den_ps0 = mbps_pool.tile([128, 8 * 2], F32, tag="den0")
                den_ps1 = mbps_pool.tile([128, 8 * 2], F32, tag="den1")
                den_ps = [den_ps0, den_ps1]
            for bi in range(NB):
                el = None
                if t > 0:
                    el = work.tile([128, 1024], BF16, tag="expl")
                    nc.scalar.activation(el[:], lps_tiles[bi][:], AF.Exp)
                for gi in range(4):
                    g = bi * 4 + gi
                    nl, j = g // J, g % J
                    out_g = mb_ps[g // 8][:, (g % 8) * 64:
                                          (g % 8) * 64 + 64]
                    for ch in range(2):
                        if t == 0:
                            lhsT = ones_bf[:]
                        else:
                            lhsT = el[:, gi * 256 + ch * 128:
                                      gi * 256 + (ch + 1) * 128]
                        if t > 0:
                            # denominator first so recd is ready by extraction
                            nc.tensor.matmul(
                                den_ps[g // 8][:, (g % 8) * 2 + ch:
                                               (g % 8) * 2 + ch + 1],
                                lhsT, onecol[:], start=True, stop=True)
                        rx = Xh[nl, ch][:].rearrange(
                            "p (a kq) -> p a kq", kq=PS)[:, 4 * j:4 * j + 4, :]
                        nc.tensor.matmul(out_g, lhsT, rx,
                                         start=(ch == 0), stop=False)
                        rxl = Xl[nl, ch][:].rearrange(
                            "p (a kq) -> p a kq",
                            kq=PS)[:, 4 * j:4 * j + 4, :]
                        nc.tensor.matmul(out_g, lhsT, rxl,
                                         start=False, stop=(ch == 1))

            # ================ post-Mb phase, pipelined per half H
            # (half H = g in [H*8, H*8+8) = local batch item nl == H, cols
            # [H*128, (H+1)*128) of all (g,kq)-shaped tensors)
            ub_halves = {}

            for H in range(2):
                gsl = slice(0, 8)
                csl = slice(0, 128)
                mbv = mb_ps[H][:].rearrange("p (g c) -> p g c", c=64)
                if t < 2:
                    mbar = state.tile([128, 8 * PS], F16, tag=f"mbar{t}{H}")
                    z = state.tile([128, 8 * PS], F32, tag=f"z{t}{H}")
                    ub = state.tile([128, 8 * PS], F16, tag=f"ubar{t}{H}")
                    uta = work.tile([PS, 8 * 128], F16, tag=f"uta{H}")
                else:
                    mbar = state.tile([128, 8 * PS], F32, tag=f"mbar32{H}")
                    s = state.tile([128, 8 * PS], F32, tag=f"s{H}")
                    outsb = state.tile([128, 8 * PS], F32, tag=f"outsb{H}")
                mview = mbar[:].rearrange("p (g kq) -> p g kq", kq=PS)

                # ---- denominators for this half
                recd = None
                if t > 0:
                    dview = den_ps[H][:].rearrange("p (g c) -> p g c",
                                                   c=2)
                    dcp = small.tile([128, 8], F32, tag=f"dcp{H}")
                    nc.vector.tensor_copy(dcp[:], dview[:, gsl, 1])
                    dsum = small.tile([128, 8], F32, tag=f"dsum{H}")
                    nc.vector.tensor_add(dsum[:], dview[:, gsl, 0], dcp[:])
                    recd = small.tile([128, 8], F32, tag=f"recd{H}")
                    nc.vector.reciprocal(recd[:], dsum[:])

                # ---- extract diagonal blocks + normalize
                for aL in range(4):
                    src_ = mbv[aL * 32:(aL + 1) * 32, gsl,
                               aL * 16:aL * 16 + 16]
                    dst_ = mview[aL * 32:(aL + 1) * 32]
                    if t == 0:
                        if aL < 2:
                            nc.vector.tensor_scalar_mul(dst_, src_, 1.0 / HW)
                        else:
                            nc.scalar.activation(dst_, src_, AF.Identity,
                                                 scale=1.0 / HW)
                    else:
                        rb = recd[aL * 32:(aL + 1) * 32].unsqueeze(2) \
                            .broadcast_to((32, 8, PS))
                        nc.vector.tensor_tensor(dst_, src_, rb, op=ALU.mult)

                if t < 2:
                    # ---- Z = G @ Mbar (fp16 elementwise + add tree)
                    tz = work.tile([128, 8 * 64], F16, tag=f"tz{H}")
                    tzv = tz[:].rearrange("p (g kp k q) -> p g kp k q",
                                          kp=4, k=4, q=4)
                    gav = GA[:].rearrange("p (g kp k q) -> p g kp k q",
                                          kp=4, k=4, q=4)[:, gsl]
                    min1 = mview.rearrange(
                        "p g (kp q) -> p g kp q", q=4) \
                        .unsqueeze(3).broadcast_to((128, 8, 4, 4, 4))
                    nc.vector.tensor_tensor(tzv, gav, min1, op=ALU.mult)
                    tzs = tz[:].rearrange("p (g kp k q) -> p kp g k q",
                                          kp=4, k=4, q=4)
                    t01 = work.tile([128, 8 * PS], F16, tag=f"t01{H}")
                    t01v = t01[:].rearrange("p (g k q) -> p g k q", k=4, q=4)
                    nc.vector.tensor_add(t01v, tzs[:, 0], tzs[:, 1])
                    t23 = work.tile([128, 8 * PS], F16, tag=f"t23{H}")
                    t23v = t23[:].rearrange("p (g k q) -> p g k q", k=4, q=4)
                    nc.vector.tensor_add(t23v, tzs[:, 2], tzs[:, 3])
                    nc.vector.tensor_add(z[:], t01[:], t23[:])
                    # ---- n2 = <Mbar, Z>
                    mz = state.tile([128, 8 * PS], F32, tag=f"mz{H}")
                    nc.vector.tensor_mul(mz[:], mbar[:], z[:])
                    n2 = small.tile([128, 8], F32, tag=f"n2{H}")
                    nc.vector.tensor_reduce(
                        out=n2[:],
                        in_=mz[:].rearrange("p (g kq) -> p g kq", kq=PS),
                        op=ALU.add, axis=AX.X)
                else:
                    # ---- final S = W @ Mbar (fp32 elementwise path)
                    ts = work.tile([128, 8 * 64], F32, tag=f"ts{H}")
                    tsv = ts[:].rearrange("p (g k pp q) -> p g k pp q",
                                          k=4, pp=4, q=4)
                    wsv = WS[:].rearrange("p (g k pp q) -> p g k pp q",
                                          k=4, pp=4, q=4)[:, gsl]
                    min2 = mview.rearrange(
                        "p g (k q) -> p g k q", q=4) \
                        .unsqueeze(3).broadcast_to((128, 8, 4, 4, 4))
                    nc.vector.tensor_tensor(tsv, wsv, min2, op=ALU.mult)
                    nc.vector.tensor_reduce(
                        out=s[:].rearrange("p (g pq) -> p g pq", pq=PS),
                        in_=ts[:].rearrange("p (g k pp q) -> p g pp q k",
                                            k=4, pp=4, q=4),
                        op=ALU.add, axis=AX.X)
                    mz = state.tile([128, 8 * PS], F32, tag=f"mz{H}")
                    nc.vector.tensor_mul(mz[:], s[:], s[:])
                    n2 = small.tile([128, 8], F32, tag=f"n2{H}")
                    nc.vector.tensor_reduce(
                        out=n2[:],
                        in_=mz[:].rearrange("p (g kq) -> p g kq", kq=PS),
                        op=ALU.add, axis=AX.X)

                # ---- squash factor f = n2/(1+n2)/sqrt(n2+eps)
                tln = small.tile([128, 8], F32, tag=f"tln{H}")
                nc.scalar.activation(tln[:], n2[:], AF.Ln, bias=epsc[:])
                rr = small.tile([128, 8], F32, tag=f"rr{H}")
                nc.scalar.activation(rr[:], tln[:], AF.Exp, scale=-0.5)
                dd = small.tile([128, 8], F32, tag=f"dd{H}")
                nc.vector.tensor_scalar_add(dd[:], n2[:], 1.0)
                rec = small.tile([128, 8], F32, tag=f"rec{H}")
                nc.vector.reciprocal(rec[:], dd[:])
                ff = small.tile([128, 8], F32, tag=f"ff{H}")
                nc.vector.tensor_mul(ff[:], n2[:], rec[:])
                ff2 = small.tile([128, 8], F32, tag=f"ff2{H}")
                nc.vector.tensor_mul(ff2[:], ff[:], rr[:])
                fbc = ff2[:].unsqueeze(2).broadcast_to((128, 8, PS))

                if t == 2:
                    # ---- output Pout = f * S; half H is local batch item H
                    nc.vector.tensor_tensor(
                        outsb[:].rearrange("p (g kq) -> p g kq", kq=PS),
                        s[:].rearrange("p (g kq) -> p g kq", kq=PS),
                        fbc, op=ALU.mult)
                    src_o = outsb[:].rearrange("p (jj kq) -> p jj kq",
                                               kq=PS)
                    dst_o = o32[H].rearrange("(jj aL) b kq -> (aL b) jj kq",
                                             jj=J)
                    nc.sync.dma_start(out=dst_o, in_=src_o)
                    continue

                # ---- U = f*Z ; Ubar += U
                ubv = ub[:].rearrange("p (g kq) -> p g kq", kq=PS)
                zv = z[:].rearrange("p (g kq) -> p g kq", kq=PS)
                if t == 0:
                    nc.vector.tensor_tensor(ubv, zv, fbc, op=ALU.mult)
                else:
                    u16 = state.tile([128, 8 * PS], F16, tag=f"u16{H}")
                    nc.vector.tensor_tensor(
                        u16[:].rearrange("p (g kq) -> p g kq", kq=PS),
                        zv, fbc, op=ALU.mult)
                    nc.vector.tensor_add(ub[:], ubar_prev[H][:],
                                         u16[:])

                # ---- UT: xbar transpose + DRAM round-trip to partition 0
                ub_halves[H] = ub
                qeng = nc.sync
                uth = work.tile([128, 128], F16, tag=f"uth{H}")
                qeng.dma_start_transpose(out=uth[:], in_=ub[:])
                udr = dram.tile([128, 128], F16, tag=f"udr{H}")
                qeng.dma_start(out=udr[:], in_=uth[:])
                qeng.dma_start(
                    out=uta[:].rearrange("p (gl ab) -> p gl ab", gl=8),
                    in_=udr[:].rearrange("(gl kq) ab -> kq gl ab", kq=16))
                ut16 = {g: uta[:, (g - H * 8) * 128:(g - H * 8 + 1) * 128]
                        for g in range(H * 8, H * 8 + 8)}

                # ---- L matmuls for next iter (this half's groups)
                for bi in (H * 2, H * 2 + 1):
                    lp = lps_pool.tile([128, 1024], F32, tag="lps")
                    lps_tiles[bi] = lp
                    for gi in range(4):
                        g = bi * 4 + gi
                        for ch in range(2):
                            for aL in range(4):
                                lhsT = MT16[g][0:PS,
                                               aL * 256 + ch * 128:
                                               aL * 256 + (ch + 1) * 128]
                                rhs = ut16[g][0:PS, aL * 32:(aL + 1) * 32]
                                nc.tensor.matmul(
                                    lp[:, gi * 256 + ch * 128 + aL * 32:
                                       gi * 256 + ch * 128 + (aL + 1) * 32],
                                    lhsT, rhs, start=True, stop=True)
            if t < 2:
                ubar_prev = ub_halves
            if _STOP == f"t{t}l":
                dmp = state.tile([128, 256], F32, tag="dmp")
                nc.vector.tensor_copy(dmp[:], lps_tiles[0][:, 0:256])
                dump(dmp[:])
                return


def _build_kernel():
    nc = bacc.Bacc("TRN2", target_bir_lowering=False, debug=False,
                   num_devices=NCORES)
    xs16t = nc.dram_tensor("xs16t", [G, PS, 4 * HW], F16,
                           kind="ExternalInput").ap()
    xh16 = nc.dram_tensor("xh16", [NL, HW, A * PS], BF16,
                          kind="ExternalInput").ap()
    xl16 = nc.dram_tensor("xl16", [NL, HW, A * PS], BF16,
                          kind="ExternalInput").ap()
    wga = nc.dram_tensor("wga", [128, G * 64], F16, kind="ExternalInput").ap()
    wws = nc.dram_tensor("wws", [128, G * 64], F32, kind="ExternalInput").ap()
    o32 = nc.dram_tensor("o32", [NL, A, B, PS], F32,
                         kind="ExternalOutput").ap()

    with tile.TileContext(nc) as tc:
        _emit(tc, xs16t, xh16, xl16, wga, wws, o32)

    nc.compile()
    return nc


# ---------------------------------------------------------------- host side
def _host_weights(weights):
    W = np.asarray(weights, np.float32)                # (A, B, P, P)
    Gm = np.einsum("abpk,abpl->abkl", W, W)            # (A, B, 4, 4): G[k, kp]
    Gsw = np.swapaxes(Gm, 2, 3)                        # Gsw[a,b,kp,k]=Gm[k,kp]
    Wsw = np.swapaxes(W, 2, 3)                         # Wsw[a,b,k,pp]=W[pp,k]

    wga = np.zeros((4, B, G, 4, 4, 4), np.float32)     # (aL,b,g,kp,k,q)
    wws = np.zeros((4, B, G, 4, 4, 4), np.float32)     # (aL,b,g,k,pp,q)
    for g in range(G):
        j = g % J                                      # g = nl*8 + j
        wga[:, :, g] = Gsw[4 * j:4 * j + 4, :, :, :, None]
        wws[:, :, g] = Wsw[4 * j:4 * j + 4, :, :, :, None]
    wga = wga.reshape(4 * B, G * 64)
    wws = wws.reshape(4 * B, G * 64)
    return wga.astype(np.float16), wws.astype(np.float32)


def _host_prep(x, weights):
    xr = np.asarray(x, np.float32).reshape(BATCH, HW, A, PS)
    wga, wws = _host_weights(weights)

    in_maps = []
    for c in range(NCORES):
        xc = xr[c * NL:(c + 1) * NL]                   # (NL, HW, A, PS)
        xh = xc.astype(ml_dtypes.bfloat16)
        xl = (xc - xh.astype(np.float32)).astype(ml_dtypes.bfloat16)
        # xs16t[g, kq, aL*256 + h] = x[nl, h, 4j+aL, kq];  g = nl*8 + j
        xj = xc.reshape(NL, HW, J, 4, PS)              # (nl,h,j,aL,kq)
        xs16t = xj.transpose(0, 2, 4, 3, 1).astype(np.float16)  # nl,j,kq,aL,h
        in_maps.append({
            "xs16t": np.ascontiguousarray(xs16t.reshape(G, PS, 4 * HW)),
            "xh16": np.ascontiguousarray(xh.reshape(NL, HW, A * PS)),
            "xl16": np.ascontiguousarray(xl.reshape(NL, HW, A * PS)),
            "wga": wga,
            "wws": wws,
        })
    return in_maps


_NC_CACHE = {}


def kernel(x, weights):
    if "nc" not in _NC_CACHE:
        _NC_CACHE["nc"] = _build_kernel()
    nc = _NC_CACHE["nc"]
    in_maps = _host_prep(x, weights)
    res = run_bass_kernel_spmd(nc, in_maps, list(range(NCORES)))
    out = np.concatenate([res.results[c]["o32"] for c in range(NCORES)],
                         axis=0)
    return out.astype(np.float32)


# revision 64
# speedup vs baseline: 1.0124x; 1.0124x over previous
"""DenseCapsLayer Trainium2 kernel.

Math (per (n, a) pair; A=32 input capsule types, B=32 output, P=4, hw=256):
  votes v[h,b] = W[a,b] @ M[h]  (4x4 matmuls) -- NEVER materialized (256MB).
  Routing reduces to small per-pair contractions:
    Mbar[b]   = sum_h c[h,b] * M[h]          (c = softmax over h of L)
    S[b]      = W[a,b] @ Mbar[b]
    n2[b]     = |S[b]|^2 = <Mbar[b], G[a,b] @ Mbar[b]>,  G = W^T W  (host-precomputed)
    Pout[b]   = f(n2) * S[b]                  (squash factor f)
    U[b]      = W^T Pout[b] = f * G @ Mbar[b]
    L        += M @ U^T  (so L_t = M @ Ubar_t^T with Ubar = cumulative sum of U)
  Final output = Pout at iter 2.

Sharding: data-parallel over batch: core c handles n in {2c, 2c+1} (NL=2), all
32 a's. Per-core layout: 16 "groups" g = j*2 + nl (j = a-block of 4, nl =
local n); partitions = (aL, b) = aL*32 + b with aL = a - 4j.
"""

import numpy as np
import ml_dtypes

import concourse.bass as bass
import concourse.bacc as bacc
import concourse.mybir as mybir
import concourse.tile as tile
from concourse.bass_utils import run_bass_kernel_spmd

F32 = mybir.dt.float32
F16 = mybir.dt.float16
BF16 = mybir.dt.bfloat16

A, B, P, ITERS = 32, 32, 4, 3
PS = P * P                      # 16
BATCH, OH, OW = 16, 16, 16
HW = OH * OW                    # 256
NCORES = 8
NL = BATCH // NCORES            # 2 local batch items per core
J = A // 4                      # 8 groups of 4 a's
G = J * NL                      # 16 (g = j*NL + nl)
NB = 4                          # g-batches for L/exp processing (4 g each)
EPS = 1e-8

AF = mybir.ActivationFunctionType
ALU = mybir.AluOpType
AX = mybir.AxisListType


# ---------------------------------------------------------------- device code
import os as _os
_STOP = _os.environ.get("K_STOP", "")


def _emit(tc, xs16t, xh16, xl16, wga, wws, o32):
    nc = tc.nc

    dbg_view = o32.rearrange("n a b k -> (n a b k)") \
                  .rearrange("(p f) -> p f", f=256)

    def dump(src):
        # debug: copy a (128, 256) fp32 AP to the output
        nc.sync.dma_start(out=dbg_view, in_=src)

    with (
        tc.tile_pool(name="inp", bufs=1) as inp,
        tc.tile_pool(name="state", bufs=1) as state,
        tc.tile_pool(name="work", bufs=3) as work,
        tc.tile_pool(name="small", bufs=2) as small,
        tc.tile_pool(name="lps", bufs=2, space="PSUM") as lps_pool,
        tc.tile_pool(name="mbps", bufs=1, space="PSUM") as mbps_pool,
        tc.tile_pool(name="dram", bufs=2, space="DRAM") as dram,
    ):
        # ---------------- persistent inputs in SBUF (batched DMAs)
        Xh = {}
        Xl = {}
        for ch in range(2):
            th = inp.tile([128, NL * A * PS], BF16, tag=f"xh{ch}")
            nc.sync.dma_start(
                out=th[:].rearrange("p (n c) -> p n c", n=NL),
                in_=xh16[:, ch * 128:(ch + 1) * 128, :].rearrange(
                    "n p c -> p n c"))
            tl = inp.tile([128, NL * A * PS], BF16, tag=f"xl{ch}")
            nc.sync.dma_start(
                out=tl[:].rearrange("p (n c) -> p n c", n=NL),
                in_=xl16[:, ch * 128:(ch + 1) * 128, :].rearrange(
                    "n p c -> p n c"))
            for nl in range(NL):
                Xh[nl, ch] = th[:, nl * A * PS:(nl + 1) * A * PS]
                Xl[nl, ch] = tl[:, nl * A * PS:(nl + 1) * A * PS]

        GA = inp.tile([128, G * 64], F16, tag="ga")
        nc.scalar.dma_start(out=GA[:], in_=wga[:, :])
        WS = inp.tile([128, G * 64], F32, tag="ws")
        nc.scalar.dma_start(out=WS[:], in_=wws[:, :])

        # MTall: (kq, g*1024 + aL*256 + h) fp16 -- M^T pre-transposed on the
        # HOST (static input), one DMA.  All matmul operands must live at
        # partition base 0 in this environment (mixing PE row-groups faults).
        MTall = inp.tile([PS, G * 4 * HW], F16, tag="mtall")
        nc.sync.dma_start(
            out=MTall[:].rearrange("p (g c) -> p g c", g=G),
            in_=xs16t.rearrange("g p c -> p g c"))
        MT16 = {g: MTall[:, g * 4 * HW:(g + 1) * 4 * HW] for g in range(G)}

        ones_bf = inp.tile([128, 128], BF16, tag="ones_bf")
        nc.gpsimd.memset(ones_bf[:], 1.0)
        onecol = inp.tile([128, 1], BF16, tag="onecol")
        nc.gpsimd.memset(onecol[:], 1.0)
        epsc = inp.tile([128, 1], F32, tag="epsc")
        nc.gpsimd.memset(epsc[:], EPS)

        # Preload the combined exp+ln activation table set once; otherwise the
        # table-load pass alternates exp_and_others / natural_log every iter
        # (~1.3us per reload).
        from concourse.hw_specs import get_activation_tables
        _tables = list(get_activation_tables(nc.m.arch).items())
        _set_id = next(i for i, (nm, fns) in enumerate(_tables)
                       if AF.Exp in fns and AF.Ln in fns)
        nc.scalar.add_instruction(mybir.InstLoadActFuncSet(
            name=nc.get_next_instruction_name(),
            ins=[], outs=[], act_func_set_id=_set_id))

        if _STOP == "setup":
            dump(WS[:, 0:256])
            return

        ubar_prev = None
        lps_tiles = {}

        for t in range(ITERS):
            # -------- Mb matmuls (+ exp for t>0), processed in 4-g batches
            mb_ps0 = mbps_pool.tile([128, 8 * 64], F32, tag="mb0")
            mb_ps1 = mbps_pool.tile([128, 8 * 64], F32, tag="mb1")
            mb_ps = [mb_ps0, mb_ps1]
            den_ps = None
            if t > 0:
                den_ps0 = mbps_pool.tile([128, 8 * 2], F32, tag="den0")
                den_ps1 = mbps_pool.tile([128, 8 * 2], F32, tag="den1")
                den_ps = [den_ps0, den_ps1]
            for bi in range(NB):
                el = None
                if t > 0:
                    el = work.tile([128, 1024], BF16, tag="expl")
                    nc.scalar.activation(el[:], lps_tiles[bi][:], AF.Exp)
                for gi in range(4):
                    g = bi * 4 + gi
                    nl, j = g // J, g % J
                    out_g = mb_ps[g // 8][:, (g % 8) * 64:
                                          (g % 8) * 64 + 64]
                    for ch in range(2):
                        if t == 0:
                            lhsT = ones_bf[:]
                        else:
                            lhsT = el[:, gi * 256 + ch * 128:
                                      gi * 256 + (ch + 1) * 128]
                        if t > 0:
                            # denominator first so recd is ready by extraction
                            nc.tensor.matmul(
                                den_ps[g // 8][:, (g % 8) * 2 + ch:
                                               (g % 8) * 2 + ch + 1],
                                lhsT, onecol[:], start=True, stop=True)
                        rx = Xh[nl, ch][:].rearrange(
                            "p (a kq) -> p a kq", kq=PS)[:, 4 * j:4 * j + 4, :]
                        nc.tensor.matmul(out_g, lhsT, rx,
                                         start=(ch == 0), stop=False)
                        rxl = Xl[nl, ch][:].rearrange(
                            "p (a kq) -> p a kq",
                            kq=PS)[:, 4 * j:4 * j + 4, :]
                        nc.tensor.matmul(out_g, lhsT, rxl,
                                         start=False, stop=(ch == 1))

            # ================ post-Mb phase, pipelined per half H
            # (half H = g in [H*8, H*8+8) = local batch item nl == H, cols
            # [H*128, (H+1)*128) of all (g,kq)-shaped tensors)
            ub_halves = {}

            for H in range(2):
                gsl = slice(0, 8)
                csl = slice(0, 128)
                mbv = mb_ps[H][:].rearrange("p (g c) -> p g c", c=64)
                if t < 2:
                    mbar = state.tile([128, 8 * PS], F16, tag=f"mbar{t}{H}")
                    z = state.tile([128, 8 * PS], F32, tag=f"z{t}{H}")
                    ub = state.tile([128, 8 * PS], F16, tag=f"ubar{t}{H}")
                    uta = work.tile([PS, 8 * 128], F16, tag=f"uta{H}")
                else:
                    mbar = state.tile([128, 8 * PS], F32, tag=f"mbar32{H}")
                    s = state.tile([128, 8 * PS], F32, tag=f"s{H}")
                    outsb = state.tile([128, 8 * PS], F32, tag=f"outsb{H}")
                mview = mbar[:].rearrange("p (g kq) -> p g kq", kq=PS)

                # ---- denominators for this half
                recd = None
                if t > 0:
                    dview = den_ps[H][:].rearrange("p (g c) -> p g c",
                                                   c=2)
                    dcp = small.tile([128, 8], F32, tag=f"dcp{H}")
                    nc.vector.tensor_copy(dcp[:], dview[:, gsl, 1])
                    dsum = small.tile([128, 8], F32, tag=f"dsum{H}")
                    nc.vector.tensor_add(dsum[:], dview[:, gsl, 0], dcp[:])
                    recd = small.tile([128, 8], F32, tag=f"recd{H}")
                    nc.vector.reciprocal(recd[:], dsum[:])

                # ---- extract diagonal blocks + normalize
                for aL in range(4):
                    src_ = mbv[aL * 32:(aL + 1) * 32, gsl,
                               aL * 16:aL * 16 + 16]
                    dst_ = mview[aL * 32:(aL + 1) * 32]
                    if t == 0:
                        if aL < 2:
                            nc.vector.tensor_scalar_mul(dst_, src_, 1.0 / HW)
                        else:
                            nc.scalar.activation(dst_, src_, AF.Identity,
                                                 scale=1.0 / HW)
                    else:
                        rb = recd[aL * 32:(aL + 1) * 32].unsqueeze(2) \
                            .broadcast_to((32, 8, PS))
                        nc.vector.tensor_tensor(dst_, src_, rb, op=ALU.mult)

                if t < 2:
                    # ---- Z = G @ Mbar (fp16 elementwise + add tree)
                    tz = work.tile([128, 8 * 64], F16, tag=f"tz{H}")
                    tzv = tz[:].rearrange("p (g kp k q) -> p g kp k q",
                                          kp=4, k=4, q=4)
                    gav = GA[:].rearrange("p (g kp k q) -> p g kp k q",
                                          kp=4, k=4, q=4)[:, gsl]
                    min1 = mview.rearrange(
                        "p g (kp q) -> p g kp q", q=4) \
                        .unsqueeze(3).broadcast_to((128, 8, 4, 4, 4))
                    nc.vector.tensor_tensor(tzv, gav, min1, op=ALU.mult)
                    tzs = tz[:].rearrange("p (g kp k q) -> p kp g k q",
                                          kp=4, k=4, q=4)
                    t01 = work.tile([128, 8 * PS], F16, tag=f"t01{H}")
                    t01v = t01[:].rearrange("p (g k q) -> p g k q", k=4, q=4)
                    nc.vector.tensor_add(t01v, tzs[:, 0], tzs[:, 1])
                    t23 = work.tile([128, 8 * PS], F16, tag=f"t23{H}")
                    t23v = t23[:].rearrange("p (g k q) -> p g k q", k=4, q=4)
                    nc.vector.tensor_add(t23v, tzs[:, 2], tzs[:, 3])
                    nc.vector.tensor_add(z[:], t01[:], t23[:])
                    # ---- n2 = <Mbar, Z>
                    mz = state.tile([128, 8 * PS], F32, tag=f"mz{H}")
                    nc.vector.tensor_mul(mz[:], mbar[:], z[:])
                    n2 = small.tile([128, 8], F32, tag=f"n2{H}")
                    nc.vector.tensor_reduce(
                        out=n2[:],
                        in_=mz[:].rearrange("p (g kq) -> p g kq", kq=PS),
                        op=ALU.add, axis=AX.X)
                else:
                    # ---- final S = W @ Mbar (fp32 elementwise path)
                    ts = work.tile([128, 8 * 64], F32, tag=f"ts{H}")
                    tsv = ts[:].rearrange("p (g k pp q) -> p g k pp q",
                                          k=4, pp=4, q=4)
                    wsv = WS[:].rearrange("p (g k pp q) -> p g k pp q",
                                          k=4, pp=4, q=4)[:, gsl]
                    min2 = mview.rearrange(
                        "p g (k q) -> p g k q", q=4) \
                        .unsqueeze(3).broadcast_to((128, 8, 4, 4, 4))
                    nc.vector.tensor_tensor(tsv, wsv, min2, op=ALU.mult)
                    nc.vector.tensor_reduce(
                        out=s[:].rearrange("p (g pq) -> p g pq", pq=PS),
                        in_=ts[:].rearrange("p (g k pp q) -> p g pp q k",
                                            k=4, pp=4, q=4),
                        op=ALU.add, axis=AX.X)
                    mz = state.tile([128, 8 * PS], F32, tag=f"mz{H}")
                    nc.vector.tensor_mul(mz[:], s[:], s[:])
                    n2 = small.tile([128, 8], F32, tag=f"n2{H}")
                    nc.vector.tensor_reduce(
                        out=n2[:],
                        in_=mz[:].rearrange("p (g kq) -> p g kq", kq=PS),
                        op=ALU.add, axis=AX.X)

                # ---- squash factor f = n2/(1+n2)/sqrt(n2+eps)
                tln = small.tile([128, 8], F32, tag=f"tln{H}")
                nc.scalar.activation(tln[:], n2[:], AF.Ln, bias=epsc[:])
                rr = small.tile([128, 8], F32, tag=f"rr{H}")
                nc.scalar.activation(rr[:], tln[:], AF.Exp, scale=-0.5)
                dd = small.tile([128, 8], F32, tag=f"dd{H}")
                nc.vector.tensor_scalar_add(dd[:], n2[:], 1.0)
                rec = small.tile([128, 8], F32, tag=f"rec{H}")
                nc.vector.reciprocal(rec[:], dd[:])
                ff = small.tile([128, 8], F32, tag=f"ff{H}")
                nc.vector.tensor_mul(ff[:], n2[:], rec[:])
                ff2 = small.tile([128, 8], F32, tag=f"ff2{H}")
                nc.vector.tensor_mul(ff2[:], ff[:], rr[:])
                fbc = ff2[:].unsqueeze(2).broadcast_to((128, 8, PS))

                if t == 2:
                    # ---- output Pout = f * S; half H is local batch item H
                    nc.vector.tensor_tensor(
                        outsb[:].rearrange("p (g kq) -> p g kq", kq=PS),
                        s[:].rearrange("p (g kq) -> p g kq", kq=PS),
                        fbc, op=ALU.mult)
                    src_o = outsb[:].rearrange("p (jj kq) -> p jj kq",
                                               kq=PS)
                    dst_o = o32[H].rearrange("(jj aL) b kq -> (aL b) jj kq",
                                             jj=J)
                    nc.sync.dma_start(out=dst_o, in_=src_o)
                    continue

                # ---- U = f*Z ; Ubar += U
                ubv = ub[:].rearrange("p (g kq) -> p g kq", kq=PS)
                zv = z[:].rearrange("p (g kq) -> p g kq", kq=PS)
                if t == 0:
                    nc.vector.tensor_tensor(ubv, zv, fbc, op=ALU.mult)
                else:
                    u16 = state.tile([128, 8 * PS], F16, tag=f"u16{H}")
                    nc.vector.tensor_tensor(
                        u16[:].rearrange("p (g kq) -> p g kq", kq=PS),
                        zv, fbc, op=ALU.mult)
                    nc.vector.tensor_add(ub[:], ubar_prev[H][:],
                                         u16[:])

                # ---- UT: xbar transpose + DRAM round-trip to partition 0
                ub_halves[H] = ub
                qeng = nc.sync
                uth = work.tile([128, 128], F16, tag=f"uth{H}")
                qeng.dma_start_transpose(out=uth[:], in_=ub[:])
                udr = dram.tile([128, 128], F16, tag=f"udr{H}")
                qeng.dma_start(out=udr[:], in_=uth[:])
                qeng.dma_start(
                    out=uta[:].rearrange("p (gl ab) -> p gl ab", gl=8),
                    in_=udr[:].rearrange("(gl kq) ab -> kq gl ab", kq=16))
                ut16 = {g: uta[:, (g - H * 8) * 128:(g - H * 8 + 1) * 128]
                        for g in range(H * 8, H * 8 + 8)}

                # ---- L matmuls for next iter (this half's groups)
                for bi in (H * 2, H * 2 + 1):
                    lp = lps_pool.tile([128, 1024], F32, tag="lps")
                    lps_tiles[bi] = lp
                    for gi in range(4):
                        g = bi * 4 + gi
                        for ch in range(2):
                            for aL in range(4):
                                lhsT = MT16[g][0:PS,
                                               aL * 256 + ch * 128:
                                               aL * 256 + (ch + 1) * 128]
                                rhs = ut16[g][0:PS, aL * 32:(aL + 1) * 32]
                                nc.tensor.matmul(
                                    lp[:, gi * 256 + ch * 128 + aL * 32:
                                       gi * 256 + ch * 128 + (aL + 1) * 32],
                                    lhsT, rhs, start=True, stop=True)
            if t < 2:
                ubar_prev = ub_halves
            if _STOP == f"t{t}l":
                dmp = state.tile([128, 256], F32, tag="dmp")
                nc.vector.tensor_copy(dmp[:], lps_tiles[0][:, 0:256])
                dump(dmp[:])
                return


def _build_kernel():
    nc = bacc.Bacc("TRN2", target_bir_lowering=False, debug=False,
                   num_devices=NCORES)
    xs16t = nc.dram_tensor("xs16t", [G, PS, 4 * HW], F16,
                           kind="ExternalInput").ap()
    xh16 = nc.dram_tensor("xh16", [NL, HW, A * PS], BF16,
                          kind="ExternalInput").ap()
    xl16 = nc.dram_tensor("xl16", [NL, HW, A * PS], BF16,
                          kind="ExternalInput").ap()
    wga = nc.dram_tensor("wga", [128, G * 64], F16, kind="ExternalInput").ap()
    wws = nc.dram_tensor("wws", [128, G * 64], F32, kind="ExternalInput").ap()
    o32 = nc.dram_tensor("o32", [NL, A, B, PS], F32,
                         kind="ExternalOutput").ap()

    with tile.TileContext(nc) as tc:
        _emit(tc, xs16t, xh16, xl16, wga, wws, o32)

    nc.compile()
    return nc


# ---------------------------------------------------------------- host side
def _host_weights(weights):
    W = np.asarray(weights, np.float32)                # (A, B, P, P)
    Gm = np.einsum("abpk,abpl->abkl", W, W)            # (A, B, 4, 4): G[k, kp]
    Gsw = np.swapaxes(Gm, 2, 3)                        # Gsw[a,b,kp,k]=Gm[k,kp]
    Wsw = np.swapaxes(W, 2, 3)                         # Wsw[a,b,k,pp]=W[pp,k]

    wga = np.zeros((4, B, G, 4, 4, 4), np.float32)     # (aL,b,g,kp,k,q)
    wws = np.zeros((4, B, G, 4, 4, 4), np.float32)     # (aL,b,g,k,pp,q)
    for g in range(G):
        j = g % J                                      # g = nl*8 + j
        wga[:, :, g] = Gsw[4 * j:4 * j + 4, :, :, :, None]
        wws[:, :, g] = Wsw[4 * j:4 * j + 4, :, :, :, None]
    wga = wga.reshape(4 * B, G * 64)
    wws = wws.reshape(4 * B, G * 64)
    return wga.astype(np.float16), wws.astype(np.float32)


def _host_prep(x, weights):
    xr = np.asarray(x, np.float32).reshape(BATCH, HW, A, PS)
    wga, wws = _host_weights(weights)

    in_maps = []
    for c in range(NCORES):
        xc = xr[c * NL:(c + 1) * NL]                   # (NL, HW, A, PS)
        xh = xc.astype(ml_dtypes.bfloat16)
        xl = (xc - xh.astype(np.float32)).astype(ml_dtypes.bfloat16)
        # xs16t[g, kq, aL*256 + h] = x[nl, h, 4j+aL, kq];  g = nl*8 + j
        xj = xc.reshape(NL, HW, J, 4, PS)              # (nl,h,j,aL,kq)
        xs16t = xj.transpose(0, 2, 4, 3, 1).astype(np.float16)  # nl,j,kq,aL,h
        in_maps.append({
            "xs16t": np.ascontiguousarray(xs16t.reshape(G, PS, 4 * HW)),
            "xh16": np.ascontiguousarray(xh.reshape(NL, HW, A * PS)),
            "xl16": np.ascontiguousarray(xl.reshape(NL, HW, A * PS)),
            "wga": wga,
            "wws": wws,
        })
    return in_maps


_NC_CACHE = {}


def kernel(x, weights):
    if "nc" not in _NC_CACHE:
        _NC_CACHE["nc"] = _build_kernel()
    nc = _NC_CACHE["nc"]
    in_maps = _host_prep(x, weights)
    res = run_bass_kernel_spmd(nc, in_maps, list(range(NCORES)))
    out = np.concatenate([res.results[c]["o32"] for c in range(NCORES)],
                         axis=0)
    return out.astype(np.float32)
